# revision 33
# baseline (speedup 1.0000x reference)
"""Trainium2 Bass kernel for nn_AttentionBlock (GroupNorm + MHA + out-proj + residual).

Sharding: pure data-parallel over batch B=16 across 8 NeuronCores (2 per core).
Each core runs the identical program on its 2 batch elements; no collectives.

Per-core pipeline (L=1024 tokens, C=512 channels, 8 heads x 64):
  1. DMA x tiles [128 tok, 512 C], PE matmul-transpose (x_chunk.T @ I) to
     x^T [C, L]; PSUM->SBUF moves ride the otherwise-idle ScalarE.
  2. GroupNorm: bn_stats per channel over L, tiny PE matmuls aggregate and
     re-broadcast per-group stats (32 groups of 16 channels), DVE affine.
  3. QKV in bf16: q,k transposed [feat, tok] (head h at partition base
     (h%2)*64 -> 2-way PE row-packing of the K=64 score matmuls); v in
     [tok, head, d|ones] layout — the appended ones block makes one matmul
     produce both attn@v (rows 0-63) and 64 softmax-denominator replicas
     (rows 64-127).
  4. Attention without max-subtraction (scores ~N(0,1); exp is safe):
     S^T = k_tile^T.T @ q^T into f32 PSUM, exp on ScalarE (scale=1/8 fused),
     [v|1]^T @ expS^T accumulated in PSUM over k-tiles.
  5. Normalize (DVE approx-reciprocal of the denominator replicas + multiply),
     bf16 out-projection, +bias, residual via accumulating DMA (x pre-copied
     into the output buffer).

The two batch elements are software-pipelined: batch 1's transpose/GN/QKV
(PE/DVE-heavy) is emitted interleaved with batch 0's attention (ScalarE-bound),
and batch 0's out-projection with batch 1's attention.
"""
import os
import sys

for _p in ("/opt/trn_rl_repo",):
    if _p not in sys.path and os.path.isdir(_p):
        sys.path.insert(0, _p)

import numpy as np

import concourse.bass as bass
import concourse.bacc as bacc
import concourse.mybir as mybir
import concourse.tile as tile
from concourse.masks import make_identity

F32 = mybir.dt.float32
F32R = mybir.dt.float32r
BF16 = mybir.dt.bfloat16

B_LOCAL = 2        # batch elements per core
L = 1024           # tokens (H*W)
C = 512            # channels
NH = 8             # heads
D = 64             # head dim
GROUPS = 32
GSIZE = C // GROUPS  # 16
EPS = 1e-5
NCHUNK = C // 128    # 4 channel chunks
NTT = L // 128       # 8 token tiles
SCALE = 1.0 / 8.0    # (1/sqrt(sqrt(64)))**2 applied inside exp


def build_attention_block(tc, ctx):
    nc = tc.nc
    AF = mybir.ActivationFunctionType
    OP = mybir.AluOpType

    x_d = nc.dram_tensor("x", [B_LOCAL, L, C], F32, kind="ExternalInput").ap()
    gamma_d = nc.dram_tensor("gamma", [C], F32, kind="ExternalInput").ap()
    beta_d = nc.dram_tensor("beta", [C], F32, kind="ExternalInput").ap()
    wq_d = nc.dram_tensor("w_qkv", [C, 3 * C], F32R, kind="ExternalInput").ap()
    bq_d = nc.dram_tensor("b_qkv", [3 * C], F32, kind="ExternalInput").ap()
    wo_d = nc.dram_tensor("w_out", [C, C], F32, kind="ExternalInput").ap()
    bo_d = nc.dram_tensor("b_out", [C], F32, kind="ExternalInput").ap()
    out_d = nc.dram_tensor("out", [B_LOCAL, L, C], F32, kind="ExternalOutput").ap()

    singles = ctx.enter_context(tc.tile_pool(name="singles", bufs=1))
    xin = ctx.enter_context(tc.tile_pool(name="xin", bufs=4))
    xbf = ctx.enter_context(tc.tile_pool(name="xbf", bufs=16))
    stgp = ctx.enter_context(tc.tile_pool(name="stgp", bufs=2))
    big = ctx.enter_context(tc.tile_pool(name="big", bufs=2))
    small = ctx.enter_context(tc.tile_pool(name="small", bufs=3))
    epool = ctx.enter_context(tc.tile_pool(name="epool", bufs=4))
    rpool = ctx.enter_context(tc.tile_pool(name="rpool", bufs=3))
    hpool = ctx.enter_context(tc.tile_pool(name="hpool", bufs=3))
    pscore = ctx.enter_context(tc.tile_pool(name="pscore", bufs=2, space="PSUM"))
    paout = ctx.enter_context(tc.tile_pool(name="paout", bufs=2, space="PSUM"))
    pmm = ctx.enter_context(tc.tile_pool(name="pmm", bufs=2, space="PSUM"))

    # ---- one-time constants ----
    identity = singles.tile([128, 128], F32)
    make_identity(nc, identity)
    identity_bf = singles.tile([128, 128], BF16)
    nc.scalar.copy(identity_bf, identity)

    # e_mat[c, g] = 1 iff c//16 == g (band built via two affine selects)
    e_mat = singles.tile([128, 8], F32)       # channel -> group indicator
    nc.gpsimd.memset(e_mat, 1.0)
    nc.gpsimd.affine_select(out=e_mat, in_=e_mat, compare_op=OP.is_ge,
                            fill=0.0, base=0, pattern=[[-GSIZE, 8]],
                            channel_multiplier=1)
    nc.gpsimd.affine_select(out=e_mat, in_=e_mat, compare_op=OP.is_ge,
                            fill=0.0, base=GSIZE - 1, pattern=[[GSIZE, 8]],
                            channel_multiplier=-1)
    e2_mat = singles.tile([8, 128], F32)      # group -> channel indicator
    nc.gpsimd.memset(e2_mat, 1.0)
    nc.gpsimd.affine_select(out=e2_mat, in_=e2_mat, compare_op=OP.is_ge,
                            fill=0.0, base=0, pattern=[[1, 128]],
                            channel_multiplier=-GSIZE)
    nc.gpsimd.affine_select(out=e2_mat, in_=e2_mat, compare_op=OP.is_ge,
                            fill=0.0, base=GSIZE - 1, pattern=[[-1, 128]],
                            channel_multiplier=GSIZE)

    wq_sb = singles.tile([128, NCHUNK, 3 * C], F32R)
    wo_sb = singles.tile([128, NCHUNK, C], BF16)
    gamma_sb = singles.tile([128, NCHUNK], F32)
    beta_sb = singles.tile([128, NCHUNK], F32)
    bqk_sb = singles.tile([128, 8], F32)      # q,k biases per [partition, fi]
    bv_bc = singles.tile([128, C], F32)       # v bias broadcast across partitions
    bo_bc = singles.tile([128, C], F32)

    def load_weights():
        nc.sync.dma_start(wq_sb, wq_d.rearrange("(o p) f -> p o f", p=128))
        for kc in range(NCHUNK):
            stg2 = stgp.tile([128, C], F32, tag="stage")
            nc.sync.dma_start(stg2, wo_d.rearrange("(o p) f -> p o f", p=128)[:, kc])
            nc.vector.tensor_copy(wo_sb[:, kc], stg2)
        nc.sync.dma_start(gamma_sb, gamma_d.rearrange("(o p) -> p o", p=128))
        nc.sync.dma_start(beta_sb, beta_d.rearrange("(o p) -> p o", p=128))
        nc.sync.dma_start(bqk_sb, bq_d[0:2 * C].rearrange("(o p) -> p o", p=128))
        nc.sync.dma_start(bv_bc, bq_d[2 * C:3 * C].partition_broadcast(128))
        nc.sync.dma_start(bo_bc, bo_d.partition_broadcast(128))

    def load_x(b):
        x_tiles = []
        for tt in range(NTT):
            xt = xin.tile([128, C], F32, tag="x_in")
            nc.sync.dma_start(xt, x_d[b, tt * 128:(tt + 1) * 128, :])
            xb = xbf.tile([128, C], BF16, tag="x_bf")
            nc.scalar.copy(xb, xt)   # bf16 weights -> 1 cyc/row transpose; ACT idle here
            x_tiles.append(xb)
        return x_tiles

    def alloc_xT():
        xT = big.tile([128, NCHUNK, L], F32R, tag="xT")
        return xT

    def stage_transpose(x_tiles, xT, ccs):
        """x^T [128, chunk, L] via PE matmul-transpose on bf16 tiles."""
        for cc in ccs:
            for half in range(2):
                tp = pmm.tile([128, 512], F32, tag="mm")
                for j in range(4):
                    tt = half * 4 + j
                    nc.tensor.matmul(
                        tp[:, j * 128:(j + 1) * 128],
                        lhsT=x_tiles[tt][:, cc * 128:(cc + 1) * 128],
                        rhs=identity_bf,
                        start=True, stop=True,
                    )
                nc.vector.tensor_copy(xT[:, cc, half * 512:(half + 1) * 512], tp)

    def stage_gn(xT):
        """GroupNorm stats + affine apply, in place on xT. Per-group reduce
        and broadcast ride tiny PE matmuls; the scalar math is batched across
        all 4 channel chunks ([?, cc, 2] tiles) to cut DVE op count."""
        mv = small.tile([128, 4, 2], F32, tag="mv")
        for cc in range(NCHUNK):
            st = small.tile([128, 2, 6], F32, tag="bnst")
            for s in range(2):
                nc.vector.bn_stats(st[:, s], xT[:, cc, s * 512:(s + 1) * 512].bitcast(F32))
            nc.vector.bn_aggr(mv[:, cc, :], st)
        sq = small.tile([128, 4, 2], F32, tag="sq")   # [mean_c, E[x^2]_c]
        nc.vector.tensor_copy(sq[:, :, 0], mv[:, :, 0])
        nc.vector.tensor_tensor(sq[:, :, 1], mv[:, :, 0], mv[:, :, 0], op=OP.mult)
        nc.vector.tensor_tensor(sq[:, :, 1], sq[:, :, 1], mv[:, :, 1], op=OP.add)
        gs = pmm.tile([8, 8], F32, tag="mm")          # per-group sums via PE
        nc.tensor.matmul(gs, lhsT=e_mat, rhs=sq.rearrange("p a b -> p (a b)"),
                         start=True, stop=True)
        gsb = small.tile([8, 4, 2], F32, tag="gsb")
        nc.vector.tensor_scalar_mul(gsb, gs.rearrange("p (a b) -> p a b", b=2),
                                    1.0 / GSIZE)      # [m_g, E[x^2]_g]
        var = small.tile([8, 4], F32, tag="var")
        nc.vector.tensor_tensor(var, gsb[:, :, 0], gsb[:, :, 0], op=OP.mult)
        nc.vector.tensor_tensor(var, gsb[:, :, 1], var, op=OP.subtract)
        nc.vector.tensor_scalar(out=var, in0=var, scalar1=float(EPS), scalar2=None,
                                op0=OP.add)
        # rstd = rsqrt(var+eps) fully on DVE (keeps ScalarE's table on Exp):
        # Quake-III seed + two Newton-Raphson steps (~1e-6 rel err)
        yi = small.tile([8, 4], mybir.dt.int32, tag="yi")
        nc.vector.tensor_scalar(out=yi, in0=var.bitcast(mybir.dt.int32),
                                scalar1=1, scalar2=None,
                                op0=OP.arith_shift_right)
        nc.vector.tensor_scalar(out=yi, in0=yi, scalar1=-1, scalar2=0x5F3759DF,
                                op0=OP.mult, op1=OP.add)
        y = yi.bitcast(F32)
        t = small.tile([8, 4], F32, tag="nrt")
        for _ in range(2):
            nc.vector.tensor_tensor(t, y, y, op=OP.mult)
            nc.vector.tensor_tensor(t, t, var, op=OP.mult)
            nc.vector.tensor_scalar(out=t, in0=t, scalar1=-0.5, scalar2=1.5,
                                    op0=OP.mult, op1=OP.add)
            nc.vector.tensor_tensor(y, y, t, op=OP.mult)
        nc.vector.tensor_copy(gsb[:, :, 1], y)        # gsb = [m_g, rstd_g]
        bc = pmm.tile([128, 8], F32, tag="mm")        # broadcast back via PE
        nc.tensor.matmul(bc, lhsT=e2_mat, rhs=gsb.rearrange("p a b -> p (a b)"),
                         start=True, stop=True)
        bc2 = bc.rearrange("p (a b) -> p a b", b=2)
        ab = small.tile([128, 4, 2], F32, tag="ab")
        nc.vector.tensor_tensor(ab[:, :, 0], bc2[:, :, 1], gamma_sb, op=OP.mult)
        nc.vector.tensor_tensor(ab[:, :, 1], bc2[:, :, 0], ab[:, :, 0], op=OP.mult)
        nc.vector.tensor_tensor(ab[:, :, 1], beta_sb, ab[:, :, 1], op=OP.subtract)
        for cc in range(NCHUNK):
            nc.vector.tensor_scalar(out=xT[:, cc, :], in0=xT[:, cc, :].bitcast(F32),
                                    scalar1=ab[:, cc, 0:1], scalar2=ab[:, cc, 1:2],
                                    op0=OP.mult, op1=OP.add)

    def alloc_qkv():
        qkT = big.tile([128, 8, L], BF16, tag="qkT")
        v_sb = big.tile([128, NTT, 8, 2 * D], BF16, tag="v")
        return qkT, v_sb

    def stage_qk(xT, qkT, fis):
        for fi in fis:
            for tb in range(2):
                ps = pmm.tile([128, 512], F32, tag="mm")
                for kc in range(NCHUNK):
                    nc.tensor.matmul(
                        ps,
                        lhsT=wq_sb[:, kc, fi * 128:(fi + 1) * 128],
                        rhs=xT[:, kc, tb * 512:(tb + 1) * 512],
                        start=(kc == 0), stop=(kc == NCHUNK - 1),
                    )
                nc.vector.tensor_scalar(
                    out=qkT[:, fi, tb * 512:(tb + 1) * 512], in0=ps,
                    scalar1=bqk_sb[:, fi:fi + 1], scalar2=None, op0=OP.add)

    def stage_v(xT, v_sb, tts):
        for tt in tts:
            nc.vector.memset(v_sb[:, tt, :, D:2 * D], 1.0)
            ps = pmm.tile([128, 512], F32, tag="mm")
            for kc in range(NCHUNK):
                nc.tensor.matmul(
                    ps,
                    lhsT=xT[:, kc, tt * 128:(tt + 1) * 128],
                    rhs=wq_sb[:, kc, 2 * C:3 * C],
                    start=(kc == 0), stop=(kc == NCHUNK - 1),
                )
            nc.vector.tensor_tensor(
                out=v_sb[:, tt, :, 0:D],
                in0=ps.rearrange("p (h d) -> p h d", d=D),
                in1=bv_bc.rearrange("p (h d) -> p h d", d=D), op=OP.add)

    def attn_block(qkT, v_sb, aT, hp, qb):
        """Attention for head pair (2*hp, 2*hp+1), query block qb; the two
        heads' K=64 score matmuls live on partition halves 0-63 / 64-127 and
        row-pack on PE."""
        h0, h1 = 2 * hp, 2 * hp + 1
        qT0 = qkT[0:64, hp, :]
        kT0 = qkT[0:64, 4 + hp, :]
        qT1 = qkT[64:128, hp, :]
        kT1 = qkT[64:128, 4 + hp, :]
        if True:
            qs = slice(qb * 512, (qb + 1) * 512)
            out0 = paout.tile([128, 512], F32, tag="aout")
            out1 = paout.tile([128, 512], F32, tag="aout")
            for g in range(4):
                s0 = pscore.tile([128, 2, 512], F32, tag="sc")
                s1 = pscore.tile([128, 2, 512], F32, tag="sc")
                for j in range(2):
                    kt = 2 * g + j
                    ks = slice(kt * 128, (kt + 1) * 128)
                    nc.tensor.matmul(s0[:, j], lhsT=kT0[:, ks], rhs=qT0[:, qs],
                                     start=True, stop=True)
                    nc.tensor.matmul(s1[:, j], lhsT=kT1[:, ks], rhs=qT1[:, qs],
                                     start=True, stop=True)
                e0 = epool.tile([128, 2, 512], BF16, tag="e")
                e1 = epool.tile([128, 2, 512], BF16, tag="e")
                nc.scalar.activation(e0, s0, AF.Exp, scale=SCALE)
                nc.scalar.activation(e1, s1, AF.Exp, scale=SCALE)
                for j in range(2):
                    kt = 2 * g + j
                    for (ops, vh, eh) in ((out0, h0, e0), (out1, h1, e1)):
                        nc.tensor.matmul(
                            ops, lhsT=v_sb[:, kt, vh, :],
                            rhs=eh[:, j], start=(kt == 0), stop=(kt == 7))
            for (ops, base) in ((out0, 0), (out1, 64)):
                den = rpool.tile([64, 512], F32, tag="den")
                nc.vector.tensor_copy(den, ops[64:128])
                rc = rpool.tile([64, 512], F32, tag="rc")
                nc.vector.reciprocal_approx_fast(rc, den)
                nc.vector.tensor_tensor(out=aT[base:base + 64, hp, qs],
                                        in0=ops[0:64], in1=rc, op=OP.mult)

    def proj_part(b, aT, tts):
        for tt in tts:
            ps = pmm.tile([128, 512], F32, tag="mm")
            for kc in range(NCHUNK):
                nc.tensor.matmul(
                    ps,
                    lhsT=aT[:, kc, tt * 128:(tt + 1) * 128],
                    rhs=wo_sb[:, kc, :],
                    start=(kc == 0), stop=(kc == NCHUNK - 1),
                )
            hh = hpool.tile([128, C], F32, tag="h")
            nc.vector.tensor_tensor(out=hh, in0=ps, in1=bo_bc, op=OP.add)
            # residual: x was pre-copied into out_d; accumulate h on top
            nc.gpsimd.dma_start(out_d[b, tt * 128:(tt + 1) * 128, :], hh,
                                accum_op=OP.add)

    # ---- schedule: software-pipeline the two batch elements ----
    # latency-critical x loads first; weights and residual pre-copies after
    xt0 = load_x(0)
    xt1 = load_x(1)
    load_weights()
    for b in range(B_LOCAL):
        nc.gpsimd.dma_start(out_d[b], x_d[b])   # residual base

    # prologue: minimum work to unlock head pair 0 of batch 0
    xT0 = alloc_xT()
    stage_transpose(xt0, xT0, range(NCHUNK))
    stage_gn(xT0)
    qkT0, v0 = alloc_qkv()
    stage_qk(xT0, qkT0, [0, 4])
    stage_v(xT0, v0, range(NTT))

    # attn(b0) qb=0 rides with the rest of qkv(b0) and transposes(b1)
    aT0 = big.tile([128, NCHUNK, L], BF16, tag="attnT")
    xT1 = alloc_xT()
    for hp in range(4):
        attn_block(qkT0, v0, aT0, hp, 0)
        if hp < 3:
            stage_qk(xT0, qkT0, [hp + 1, hp + 5])
        stage_transpose(xt1, xT1, [hp])

    # attn(b0) qb=1 rides with gn(b1) + qkv(b1)
    qkT1, v1 = alloc_qkv()
    for hp in range(4):
        attn_block(qkT0, v0, aT0, hp, 1)
        if hp == 0:
            stage_gn(xT1)
            stage_qk(xT1, qkT1, [0, 4])
        elif hp == 1:
            stage_v(xT1, v1, range(NTT))
        elif hp == 2:
            stage_qk(xT1, qkT1, [1, 5, 2, 6])
        else:
            stage_qk(xT1, qkT1, [3, 7])

    # attn(b1) qb=0 rides with proj(b0)
    aT1 = big.tile([128, NCHUNK, L], BF16, tag="attnT")
    for hp in range(4):
        attn_block(qkT1, v1, aT1, hp, 0)
        proj_part(0, aT0, range(2 * hp, 2 * hp + 2))
    # attn(b1) qb=1 rides with proj(b1) tts 0-3 (q tokens 0-511 final)
    for hp in range(4):
        attn_block(qkT1, v1, aT1, hp, 1)
        proj_part(1, aT1, [hp])
    proj_part(1, aT1, range(4, NTT))


_NC_CACHE = None


def _get_nc():
    global _NC_CACHE
    if _NC_CACHE is None:
        from contextlib import ExitStack

        nc = bacc.Bacc("TRN2", target_bir_lowering=False, debug=False)
        with tile.TileContext(nc) as tc, ExitStack() as ctx:
            build_attention_block(tc, ctx)
        nc.compile()
        _NC_CACHE = nc
    return _NC_CACHE


def run(inputs, trace=False, tmpdir=None):
    """Run on 8 NeuronCores. Returns (full_output, BassKernelResults)."""
    from concourse import bass_utils

    x = np.ascontiguousarray(np.asarray(inputs["x"], dtype=np.float32))
    B, H, W, Cc = x.shape
    xs = x.reshape(B, H * W, Cc)
    common = {
        "gamma": np.ascontiguousarray(np.asarray(inputs["gamma"], np.float32)),
        "beta": np.ascontiguousarray(np.asarray(inputs["beta"], np.float32)),
        "w_qkv": np.ascontiguousarray(np.asarray(inputs["w_qkv"], np.float32)),
        "b_qkv": np.ascontiguousarray(np.asarray(inputs["b_qkv"], np.float32)),
        "w_out": np.ascontiguousarray(np.asarray(inputs["w_out"], np.float32)),
        "b_out": np.ascontiguousarray(np.asarray(inputs["b_out"], np.float32)),
    }
    n_cores = 8
    per = B // n_cores
    in_maps = [
        {"x": np.ascontiguousarray(xs[c * per:(c + 1) * per]), **common}
        for c in range(n_cores)
    ]
    nc = _get_nc()
    res = bass_utils.run_bass_kernel_spmd(
        nc, in_maps, core_ids=list(range(n_cores)), trace=trace, tmpdir=tmpdir)
    out = np.concatenate([r["out"] for r in res.results], axis=0)
    return out.reshape(B, H, W, Cc), res


def kernel(**inputs):
    out, _ = run(inputs, trace=False)
    return out
